# revision 14
# baseline (speedup 1.0000x reference)
"""Trainium2 Bass kernel for the DIST loss (inter spearman-variant + intra
pearson).

Contract: kernel(z_s, z_t) -> scalar np.float32 () matching
reference.reference.

Strategy (8 cores, batch-sharded 512 rows/core), v2 (optimized):
  - Each element of z read from HBM once; u = exp(z - 2) in f16.
  - Rank counts r_c = #{k: u_k < u_c} for c in 0..9 via DVE tensor_scalar
    is_lt with f32 per-partition scalar + accum_out -- runs in the DVE 4x
    perf mode (0.26 ns/elem) -- split with ACT sign passes.
  - argmax via two tournaments of 2x-mode tensor_tensor max folds:
    natural blocks of 250 -> M_N [P,64]; stride-combs of 250 -> T_glob
    [P,250]; j = 250*argmax(M_N) + argmax(T_glob)  (exact).
  - Intra stats (per-class sums of y, y^2, y_s*y_t over batch) via PE
    matmuls: stationary = data chunk [128,125], moving = per-strip scaled
    weight column; per-strip PSUM -> SBUF accumulate; ReduceScatter
    across cores; per-rank pearson shard; tiny AllReduce for the scalars.
"""

import sys

import numpy as np

sys.path.insert(0, "/opt/trn_rl_repo")

# ---------------------------------------------------------------- constants
B_FULL = 4096
C = 16000
N_CORES = 8
RPC = B_FULL // N_CORES  # rows per core = 512
P = 128
NSTRIP = RPC // P        # 4
QB = 4000                # quarter block (compares/tournaments)
NQ = C // QB             # 4
ZB = 2000                # dma/exp column block
NZB = C // ZB            # 8
MP = 125                 # stationary chunk classes
NCH = C // MP            # 128 chunks
NSTATS = 5
EXP_BIAS = 2.0
RANK_CLAMP = 10
EPS = 1e-8
A_SC = 2.0 ** 8          # scale on 1/S weights (f16 headroom)
B_SC = 2.0 ** 16

# engine per (input, class): 'v' = DVE tensor_scalar, 'a' = ACT sign.
# Measured on HW: DVE tensor_scalar+accum runs 1x (4.38us/[P,4000]) -- the
# CACHE_REDUCE variant gets no fast mode -- while ACT sign is 3.72us.
# Balance: 12 units ACT, 8 DVE.
CMP_ENGINE = {}
for _t in range(2):
    for _c in range(RANK_CLAMP):
        CMP_ENGINE[(_t, _c)] = "a" if _c >= 4 else "v"
N_ACT_UNITS = sum(1 for v in CMP_ENGINE.values() if v == "a")  # 12


def build_program(rpc=RPC, c=C, n_cores=N_CORES):
    import concourse.bass as bass
    import concourse.mybir as mybir
    import concourse.tile as tile
    from concourse import bacc
    from concourse.alu_op_type import AluOpType as OP

    f32 = mybir.dt.float32
    f16 = mybir.dt.float16
    bf16 = mybir.dt.bfloat16
    u32 = mybir.dt.uint32
    ACT = mybir.ActivationFunctionType
    AX = mybir.AxisListType

    nstrip = rpc // P
    inv_n = 1.0 / (c - 1)

    nc = bacc.Bacc(None, target_bir_lowering=False, debug=False,
                   num_devices=n_cores)

    z_s = nc.declare_dram_parameter("z_s", [rpc, c], f32, isOutput=False)
    z_t = nc.declare_dram_parameter("z_t", [rpc, c], f32, isOutput=False)
    out = nc.declare_dram_parameter("out", [1, 1], f32, isOutput=True)

    def bcast(ap, dims):
        return bass.AP(tensor=ap.tensor, offset=ap.offset,
                       ap=[ap.ap[0]] + dims)

    from contextlib import ExitStack
    with tile.TileContext(nc) as tc, ExitStack() as ctx:
        zpool = ctx.enter_context(tc.tile_pool(name="zpool", bufs=2))
        upool = ctx.enter_context(tc.tile_pool(name="upool", bufs=2))
        sqpool = ctx.enter_context(tc.tile_pool(name="sqpool", bufs=1))
        tpool = ctx.enter_context(tc.tile_pool(name="tpool", bufs=1))
        small = ctx.enter_context(tc.tile_pool(name="small", bufs=1))
        stiny = ctx.enter_context(tc.tile_pool(name="stiny", bufs=2))
        psum = ctx.enter_context(tc.tile_pool(name="psum", bufs=2,
                                              space="PSUM"))
        dram = ctx.enter_context(tc.tile_pool(name="dram", bufs=1,
                                              space="DRAM"))

        # ---------------- persistent tiles ----------------
        scr_v = small.tile([P, QB], f16, tag="scr_v")
        scr_a = small.tile([P, QB], f16, tag="scr_a")
        cnt = small.tile([P, nstrip, 2, RANK_CLAMP, NQ], f32, tag="cnt")
        ssum = small.tile([P, nstrip, 2, NZB], f32, tag="ssum")
        thetas = small.tile([P, nstrip, 2, RANK_CLAMP], f32, tag="thetas")
        bt = small.tile([P, nstrip, 2, 2], f32, tag="bt")  # [b*, t*]
        T_glob = small.tile([P, nstrip, 2, 250], f16, tag="T_glob")
        M_N = small.tile([P, nstrip, 2, 64], f16, tag="M_N")
        stats_acc = small.tile([MP, NSTATS, NCH], f32, tag="stats_acc")
        nc.vector.memset(stats_acc[:], 0.0)
        nbias = small.tile([P, 1], f32, tag="nbias")
        nc.vector.memset(nbias[:], -EXP_BIAS)
        ones_col = small.tile([P, 1], f32, tag="ones_col")
        nc.vector.memset(ones_col[:], 1.0)

        cc_in = dram.tile([P, NSTATS * NCH], f32, tag="cc_in")
        cc_out = dram.tile([16, NSTATS * NCH], f32, tag="cc_out")
        ar_in = dram.tile([1, 4], f32, tag="ar_in")
        ar_out = dram.tile([1, 4], f32, tag="ar_out")

        # zero pad rows 125..127 of cc_in once
        zpad = small.tile([3, NSTATS * NCH], f32, tag="zpad")
        nc.vector.memset(zpad[:], 0.0)
        nc.sync.dma_start(out=cc_in[MP:P, :], in_=zpad[:])

        # =================== main strip loop ===================
        for s in range(nstrip):
            uq_s = []
            uq_t = []
            # ---- DMA + exp (per z-block), u quarters f16
            for t, zp in ((0, z_s), (1, z_t)):
                for q in range(NQ):
                    uq = upool.tile([P, QB], f16, tag=f"u{t}_{q}",
                                    bufs=2 if q < 3 else 1)
                    (uq_s if t == 0 else uq_t).append(uq)
                    for h in range(QB // ZB):
                        zb = zpool.tile([P, ZB], f32, tag="zb")
                        col0 = q * QB + h * ZB
                        nc.sync.dma_start(
                            out=zb[:],
                            in_=zp[s * P:(s + 1) * P, col0:col0 + ZB])
                        nc.scalar.activation(
                            uq[:, h * ZB:(h + 1) * ZB], zb[:], ACT.Exp,
                            bias=nbias[:], scale=1.0,
                            accum_out=ssum[:, s, t, q * (QB // ZB) + h:
                                           q * (QB // ZB) + h + 1])

            # ---- thetas from quarter 0 (classes 0..9 live in cols 0..9)
            for t, uql in ((0, uq_s), (1, uq_t)):
                for cc_ in range(RANK_CLAMP):
                    nc.vector.tensor_copy(thetas[:, s, t, cc_:cc_ + 1],
                                          uql[0][:, cc_:cc_ + 1])

            # ---- compares (20 units x 4 quarters)
            for t, uql in ((0, uq_s), (1, uq_t)):
                for cc_ in range(RANK_CLAMP):
                    th = thetas[:, s, t, cc_:cc_ + 1]
                    for q in range(NQ):
                        acc = cnt[:, s, t, cc_, q:q + 1]
                        if CMP_ENGINE[(t, cc_)] == "v":
                            nc.vector.tensor_scalar(
                                scr_v[:], uql[q][:], th, 0.0,
                                OP.is_lt, OP.add, accum_out=acc)
                        else:
                            nc.scalar.activation(
                                scr_a[:], uql[q][:], ACT.Sign,
                                bias=th, scale=-1.0, accum_out=acc)

            # ---- argmax tournaments per (tensor, quarter)
            for t, uql in ((0, uq_s), (1, uq_t)):
                for q in range(NQ):
                    uq = uql[q]
                    # T-fold: contiguous halving 4000->250
                    ta = tpool.tile([P, 2000], f16, tag="ta")
                    tb = tpool.tile([P, 1000], f16, tag="tb")
                    nc.vector.tensor_tensor(ta[:, 0:2000], uq[:, 0:2000],
                                            uq[:, 2000:4000], OP.max)
                    nc.vector.tensor_tensor(tb[:, 0:1000], ta[:, 0:1000],
                                            ta[:, 1000:2000], OP.max)
                    nc.vector.tensor_tensor(ta[:, 0:500], tb[:, 0:500],
                                            tb[:, 500:1000], OP.max)
                    tg = T_glob[:, s, t, :]
                    if q == 0:
                        nc.vector.tensor_tensor(tg, ta[:, 0:250],
                                                ta[:, 250:500], OP.max)
                    else:
                        nc.vector.tensor_tensor(tb[:, 0:250], ta[:, 0:250],
                                                ta[:, 250:500], OP.max)
                        nc.vector.tensor_tensor(tg, tg, tb[:, 0:250], OP.max)
                    # N-fold: within blocks of 250: [P, 16, 250] -> [P, 16]
                    vN = uq[:].rearrange("p (b i) -> p b i", b=16)
                    na = tpool.tile([P, 16, 125], f16, tag="na")
                    nb = tpool.tile([P, 16, 63], f16, tag="nb")
                    nc.vector.tensor_tensor(na[:], vN[:, :, 0:125],
                                            vN[:, :, 125:250], OP.max)
                    srcN, curn = na, 125
                    dsts = [nb, na]
                    di = 0
                    while curn > 1:
                        half = (curn + 1) // 2
                        dst = dsts[di % 2]
                        nc.vector.tensor_tensor(
                            dst[:, :, 0:half], srcN[:, :, 0:half],
                            srcN[:, :, curn - half:curn], OP.max)
                        srcN, curn, di = dst, half, di + 1
                    nc.vector.tensor_copy(
                        M_N[:, s, t, q * 16:(q + 1) * 16], srcN[:, :, 0])

            # ---- argmax finalize per tensor
            for t in range(2):
                m8 = stiny.tile([P, 8], f16, tag="m8")
                i8 = stiny.tile([P, 8], u32, tag="i8")
                nc.vector.max(m8[:], M_N[:, s, t, :])
                m8b = bcast(m8[:, 0:1], [[0, 8]])
                nc.vector.max_index(i8[:], m8b, M_N[:, s, t, :])
                nc.vector.tensor_copy(bt[:, s, t, 0:1], i8[:, 0:1])
                nc.vector.max_index(i8[:], m8b, T_glob[:, s, t, :])
                nc.vector.tensor_copy(bt[:, s, t, 1:2], i8[:, 0:1])

            # ---- weights from row sums
            sS = stiny.tile([P, 2], f32, tag="sS")
            nc.vector.reduce_sum(sS[:, 0:1], ssum[:, s, 0, :], axis=AX.X)
            nc.vector.reduce_sum(sS[:, 1:2], ssum[:, s, 1, :], axis=AX.X)
            rr = stiny.tile([P, 2], f32, tag="rr")
            nc.vector.reciprocal(rr[:], sS[:])
            wf = stiny.tile([P, NSTATS], f32, tag="wf")
            # w_a = A_SC * r_s ; w_c = A_SC * r_t
            nc.vector.tensor_scalar(wf[:, 0:1], rr[:, 0:1], A_SC, None,
                                    OP.mult)
            nc.vector.tensor_scalar(wf[:, 2:3], rr[:, 1:2], A_SC, None,
                                    OP.mult)
            # w_b = B_SC * r_s^2 ; w_d = B_SC * r_t^2 ; w_e = B_SC*r_s*r_t
            r2 = stiny.tile([P, 3], f32, tag="r2")
            nc.vector.tensor_tensor(r2[:, 0:1], rr[:, 0:1], rr[:, 0:1],
                                    OP.mult)
            nc.vector.tensor_tensor(r2[:, 1:2], rr[:, 1:2], rr[:, 1:2],
                                    OP.mult)
            nc.vector.tensor_tensor(r2[:, 2:3], rr[:, 0:1], rr[:, 1:2],
                                    OP.mult)
            nc.vector.tensor_scalar(wf[:, 1:2], r2[:, 0:1], B_SC, None,
                                    OP.mult)
            nc.vector.tensor_scalar(wf[:, 3:4], r2[:, 1:2], B_SC, None,
                                    OP.mult)
            nc.vector.tensor_scalar(wf[:, 4:5], r2[:, 2:3], B_SC, None,
                                    OP.mult)
            wcol = stiny.tile([P, NSTATS], f16, tag="wcol")
            nc.vector.tensor_copy(wcol[:], wf[:])

            # ---- squares / cross (strip end), consumed by PE
            stats_ps = psum.tile([MP, NSTATS, NCH], f32, tag="stats_ps")
            for q in range(NQ):
                sq_s = sqpool.tile([P, QB], bf16, tag="sq_s")
                sq_t = sqpool.tile([P, QB], bf16, tag="sq_t")
                xst = sqpool.tile([P, QB], bf16, tag="xst")
                nc.gpsimd.tensor_tensor(sq_s[:], uq_s[q][:], uq_s[q][:],
                                        OP.mult)
                nc.gpsimd.tensor_tensor(sq_t[:], uq_t[q][:], uq_t[q][:],
                                        OP.mult)
                nc.gpsimd.tensor_tensor(xst[:], uq_s[q][:], uq_t[q][:],
                                        OP.mult)
                # stats matmuls for this quarter's chunks
                for k in range(QB // MP):  # 32 chunks / quarter
                    kk = q * (QB // MP) + k
                    ksl = slice(k * MP, (k + 1) * MP)
                    lhss = ((uq_s[q][:, ksl], 0), (sq_s[:, ksl], 1),
                            (uq_t[q][:, ksl], 2), (sq_t[:, ksl], 3),
                            (xst[:, ksl], 4))
                    for lhsT, si in lhss:
                        nc.tensor.matmul(
                            stats_ps[0:MP, si, kk:kk + 1],
                            lhsT, wcol[:, si:si + 1],
                            start=True, stop=True)
            nc.vector.tensor_tensor(stats_acc[:], stats_acc[:],
                                    stats_ps[0:MP, :, :], OP.add)

        # ================= inter-term combine =================
        # js/jt = 250*b + t
        js = small.tile([P, nstrip], f32, tag="js")
        jt = small.tile([P, nstrip], f32, tag="jt")
        for t, jx in ((0, js), (1, jt)):
            nc.vector.tensor_scalar(jx[:], bt[:, :, t, 0], 250.0, None,
                                    OP.mult)
            nc.vector.tensor_tensor(jx[:], jx[:], bt[:, :, t, 1], OP.add)

        # counts: reduce over quarters
        cr = small.tile([P, nstrip, 2, RANK_CLAMP, 1], f32, tag="cr")
        nc.vector.reduce_sum(cr[:], cnt[:], axis=AX.X)
        # ACT units: count = 0.5*acc + (c - 1)/2 + 0.5*c/2?? ->
        # count = 0.5*acc + 0.5*(c_total - 1): sign sum over all c elems,
        # self term contributes 0 and is the single "eq".
        for t in range(2):
            for cc_ in range(RANK_CLAMP):
                if CMP_ENGINE[(t, cc_)] == "a":
                    v = cr[:, :, t, cc_, 0]
                    nc.vector.tensor_scalar(v, v, float(c - 1), 0.5,
                                            OP.add, OP.mult)

        crs2 = cr[:, :, 0, :, 0]  # [P, strip, 10]
        crt2 = cr[:, :, 1, :, 0]
        js_b = bcast(js[:], [[1, nstrip], [0, RANK_CLAMP]])
        jt_b = bcast(jt[:], [[1, nstrip], [0, RANK_CLAMP]])
        gt_s = small.tile([P, nstrip, 10], f32, tag="gt_s")
        gt_t = small.tile([P, nstrip, 10], f32, tag="gt_t")
        kp_s = small.tile([P, nstrip, 10], f32, tag="kp_s")
        kp_t = small.tile([P, nstrip, 10], f32, tag="kp_t")
        p_s = small.tile([P, nstrip, 10], f32, tag="p_s")
        p_t = small.tile([P, nstrip, 10], f32, tag="p_t")
        for crx, jb, gt, kp, px, sent in (
                (crs2, js_b, gt_s, kp_s, p_s, 5.0),
                (crt2, jt_b, gt_t, kp_t, p_t, 7.0)):
            nc.vector.tensor_tensor(gt[:], crx, jb, OP.is_gt)
            nc.vector.tensor_tensor(kp[:], crx, jb, OP.not_equal)
            nc.vector.tensor_tensor(px[:], crx, gt[:], OP.subtract)
            nc.vector.tensor_scalar_add(px[:], px[:], sent)
            nc.vector.tensor_tensor(px[:], px[:], kp[:], OP.mult)
            nc.vector.tensor_scalar_add(px[:], px[:], -sent)

        wa = small.tile([P, 10], f32, tag="wa")
        for cc_ in range(RANK_CLAMP):
            nc.vector.memset(wa[:, cc_:cc_ + 1], float(cc_ - RANK_CLAMP))
        wa_b = bcast(wa[:], [[0, nstrip], [1, 10]])
        kw_s = small.tile([P, nstrip, 10], f32, tag="kw_s")
        kw_t = small.tile([P, nstrip, 10], f32, tag="kw_t")
        nc.vector.tensor_tensor(kw_s[:], kp_s[:], wa_b, OP.mult)
        nc.vector.tensor_tensor(kw_t[:], kp_t[:], wa_b, OP.mult)
        s1_s = small.tile([P, nstrip, 1], f32, tag="s1_s")
        s1_t = small.tile([P, nstrip, 1], f32, tag="s1_t")
        nc.vector.reduce_sum(s1_s[:], kw_s[:], axis=AX.X)
        nc.vector.reduce_sum(s1_t[:], kw_t[:], axis=AX.X)
        k2_s = small.tile([P, nstrip, 10], f32, tag="k2_s")
        k2_t = small.tile([P, nstrip, 10], f32, tag="k2_t")
        nc.vector.tensor_tensor(k2_s[:], kw_s[:], wa_b, OP.mult)
        nc.vector.tensor_tensor(k2_t[:], kw_t[:], wa_b, OP.mult)
        s2_s = small.tile([P, nstrip, 1], f32, tag="s2_s")
        s2_t = small.tile([P, nstrip, 1], f32, tag="s2_t")
        nc.vector.reduce_sum(s2_s[:], k2_s[:], axis=AX.X)
        nc.vector.reduce_sum(s2_t[:], k2_t[:], axis=AX.X)

        w100 = small.tile([P, 100], f32, tag="w100")
        nc.vector.tensor_tensor(
            w100[:],
            bcast(wa[:], [[1, 10], [0, 10]]),
            bcast(wa[:], [[0, 10], [1, 10]]), OP.mult)
        eq = small.tile([P, nstrip, 10, 10], f32, tag="eq")
        nc.vector.tensor_tensor(
            eq[:],
            bcast(p_s[:], [[10, nstrip], [1, 10], [0, 10]]),
            bcast(p_t[:], [[10, nstrip], [0, 10], [1, 10]]), OP.is_equal)
        nc.vector.tensor_tensor(
            eq[:], eq[:],
            bcast(w100[:], [[0, nstrip], [10, 10], [1, 10]]), OP.mult)
        xterm = small.tile([P, nstrip, 1, 1], f32, tag="xterm")
        nc.vector.reduce_sum(xterm[:], eq[:], axis=AX.XY)

        x2 = xterm[:, :, 0, 0]
        num = small.tile([P, nstrip], f32, tag="num")
        nc.vector.tensor_tensor(num[:], s1_s[:, :, 0], s1_t[:, :, 0], OP.mult)
        nc.vector.scalar_tensor_tensor(
            num[:], num[:], -inv_n, x2, OP.mult, OP.add)
        var_s = small.tile([P, nstrip], f32, tag="var_s")
        var_t = small.tile([P, nstrip], f32, tag="var_t")
        for s1x, s2x, varx in ((s1_s, s2_s, var_s), (s1_t, s2_t, var_t)):
            nc.vector.tensor_tensor(varx[:], s1x[:, :, 0], s1x[:, :, 0],
                                    OP.mult)
            nc.vector.scalar_tensor_tensor(
                varx[:], varx[:], -inv_n, s2x[:, :, 0], OP.mult, OP.add)
        den = small.tile([P, nstrip], f32, tag="den")
        nc.vector.tensor_tensor(den[:], var_s[:], var_t[:], OP.mult)
        nc.scalar.activation(den[:], den[:], ACT.Sqrt)
        nc.vector.tensor_scalar_add(den[:], den[:], EPS)
        nc.vector.reciprocal(den[:], den[:])
        rho = small.tile([P, nstrip], f32, tag="rho")
        nc.vector.tensor_tensor(rho[:], num[:], den[:], OP.mult)
        eqj = small.tile([P, nstrip], f32, tag="eqj")
        nc.vector.tensor_tensor(eqj[:], js[:], jt[:], OP.is_equal)

        packed = small.tile([P, 2], f32, tag="packed")
        nc.vector.reduce_sum(packed[:, 0:1], rho[:], axis=AX.X)
        nc.vector.reduce_sum(packed[:, 1:2], eqj[:], axis=AX.X)
        inter_ps = psum.tile([1, 2], f32, tag="inter_ps")
        nc.tensor.matmul(inter_ps[:], ones_col[:], packed[:],
                         start=True, stop=True)
        inter_sb = small.tile([1, 2], f32, tag="inter_sb")
        nc.vector.tensor_copy(inter_sb[:], inter_ps[:])

        # ================= stats collective =================
        nc.sync.dma_start(out=cc_in[0:MP, :],
                          in_=stats_acc[:].rearrange("p a b -> p (a b)"))
        nc.gpsimd.collective_compute(
            "ReduceScatter", OP.add,
            replica_groups=[list(range(n_cores))],
            ins=[cc_in[:].opt()], outs=[cc_out[:].opt()])

        # per-rank pearson shard: [16, 5, 128]
        sh = small.tile([16, NSTATS, NCH], f32, tag="sh")
        nc.sync.dma_start(out=sh[:].rearrange("p a b -> p (a b)"),
                          in_=cc_out[:])
        a_s, b_s, a_t, b_t, e_st = (sh[:, i, :] for i in range(NSTATS))
        inv_b = 1.0 / (rpc * n_cores)
        num2 = small.tile([16, NCH], f32, tag="num2")
        nc.vector.tensor_tensor(num2[:], a_s, a_t, OP.mult)
        nc.vector.scalar_tensor_tensor(
            num2[:], num2[:], -inv_b, e_st, OP.mult, OP.add)
        va = small.tile([16, NCH], f32, tag="va")
        vb = small.tile([16, NCH], f32, tag="vb")
        for ax, bx, vx in ((a_s, b_s, va), (a_t, b_t, vb)):
            nc.vector.tensor_tensor(vx[:], ax, ax, OP.mult)
            nc.vector.scalar_tensor_tensor(
                vx[:], vx[:], -inv_b, bx, OP.mult, OP.add)
        den2 = small.tile([16, NCH], f32, tag="den2")
        nc.vector.tensor_tensor(den2[:], va[:], vb[:], OP.mult)
        nc.scalar.activation(den2[:], den2[:], ACT.Sqrt)
        nc.vector.tensor_scalar_add(den2[:], den2[:], EPS)
        nc.vector.reciprocal(den2[:], den2[:])
        nc.vector.tensor_tensor(num2[:], num2[:], den2[:], OP.mult)
        rho_cls = small.tile([16, 1], f32, tag="rho_cls")
        nc.vector.reduce_sum(rho_cls[:], num2[:], axis=AX.X)
        intra_ps = psum.tile([1, 1], f32, tag="intra_ps")
        nc.tensor.matmul(intra_ps[:], ones_col[0:16, :], rho_cls[:],
                         start=True, stop=True)

        # tiny AllReduce: [intra_shard, rho_sum, eq_sum, 0]
        sc4 = small.tile([1, 4], f32, tag="sc4")
        nc.vector.memset(sc4[:], 0.0)
        nc.vector.tensor_copy(sc4[:, 0:1], intra_ps[:])
        nc.vector.tensor_copy(sc4[:, 1:3], inter_sb[:])
        nc.sync.dma_start(out=ar_in[:], in_=sc4[:])
        nc.gpsimd.collective_compute(
            "AllReduce", OP.add,
            replica_groups=[list(range(n_cores))],
            ins=[ar_in[:].opt()], outs=[ar_out[:].opt()])
        scf = small.tile([1, 4], f32, tag="scf")
        nc.sync.dma_start(out=scf[:], in_=ar_out[:])

        # fin = 2 - (rho_sum + eq_sum)/B - intra_sum/C
        fin = small.tile([1, 1], f32, tag="fin")
        nc.vector.tensor_tensor(fin[:], scf[:, 1:2], scf[:, 2:3], OP.add)
        nc.vector.tensor_scalar_mul(fin[:], fin[:], -inv_b)
        nc.vector.scalar_tensor_tensor(
            fin[:], scf[:, 0:1], -1.0 / c, fin[:], OP.mult, OP.add)
        nc.vector.tensor_scalar_add(fin[:], fin[:], 2.0)
        nc.sync.dma_start(out=out[:], in_=fin[:])

    nc.finalize()
    return nc


_CACHED = {}


def _get_program():
    if "nc" not in _CACHED:
        _CACHED["nc"] = build_program()
    return _CACHED["nc"]


def kernel(z_s: np.ndarray, z_t: np.ndarray) -> np.ndarray:
    from concourse.bass_utils import run_bass_kernel_spmd

    nc = _get_program()
    in_maps = []
    for i in range(N_CORES):
        sl = slice(i * RPC, (i + 1) * RPC)
        in_maps.append({
            "z_s": np.ascontiguousarray(z_s[sl], dtype=np.float32),
            "z_t": np.ascontiguousarray(z_t[sl], dtype=np.float32),
        })
    res = run_bass_kernel_spmd(nc, in_maps, core_ids=list(range(N_CORES)))
    val = np.asarray(res.results[0]["out"], dtype=np.float32).reshape(())
    return val
